# revision 5
# baseline (speedup 1.0000x reference)
"""NFM (Neural Factorization Machine) forward pass on 8 Trainium2 cores.

Full-input contract: kernel(**inputs) takes the unsharded numpy inputs and
returns the full [131072, 1] output. Internally: pure data parallelism,
batch split 8 ways, weights replicated.

Math (per reference):
    linear = x @ w_wide + b_wide
    xv     = x @ V
    x2v2   = (x*x) @ (V*V)
    tmp    = 0.5 * (xv^2 + x2v2)
    tmp    = relu(tmp @ w1 + b1); relu(tmp @ w2 + b2); relu(tmp @ w3 + b3)
    out    = sigmoid(linear + tmp @ w_out + b_out)

Device layout: everything transposed (features on partitions, batch on the
free axis), so weights act as the matmul stationary operand in natural
[in, out] layout and x is transposed on the host.

Host-side algebraic folds:
  w1' = 0.5 * w1 (absorbs the 0.5)
  W   = (V*V) @ w1'  -> layer-1 input becomes xv^2 @ w1' + x^2 @ W + b1,
        which removes the whole x2v2 GEMM + the tmp adds on device.
  bo  = b_wide + b_out (final sigmoid bias)

Matmul operands use native float32r tensors (full-rate PE at N=512).
"""

import os
from contextlib import ExitStack

import numpy as np

import concourse.bass as bass
import concourse.mybir as mybir
import concourse.tile as tile
from concourse import bacc
from concourse.bass_utils import run_bass_kernel_spmd

F = 256          # input features
KDIM = 256       # FM cross dim
D1, D2, D3 = 128, 85, 64
BATCH = 131072
NCORES = 8
BL = BATCH // NCORES   # rows per core
BT = 512               # batch tile (free dim of matmuls)

f32 = mybir.dt.float32
f32r = mybir.dt.float32r
AF = mybir.ActivationFunctionType
Alu = mybir.AluOpType


def build_program(bl=BL, bt=BT, num_devices=NCORES, reps=1):
    nc = bacc.Bacc(
        "TRN2", target_bir_lowering=False, debug=False, num_devices=num_devices
    )
    dt_in = [
        ("xT", [F, bl], f32r),
        ("V", [F, KDIM], f32r),
        ("Wf", [F, D1], f32r),     # (V*V) @ (0.5*w1)
        ("w1", [KDIM, D1], f32r),  # 0.5*w1
        ("b1", [D1, 1], f32),
        ("w2", [D1, D2], f32r),
        ("b2", [D2, 1], f32),
        ("w3", [D2, D3], f32r),
        ("b3", [D3, 1], f32),
        ("wout", [D3, 1], f32r),
        ("wwide", [F, 1], f32r),
        ("bo", [1, 1], f32),
    ]
    d = {n: nc.dram_tensor(n, s, t, kind="ExternalInput").ap() for n, s, t in dt_in}
    outT = nc.dram_tensor("outT", [1, bl], f32, kind="ExternalOutput").ap()

    with tile.TileContext(nc) as tc, ExitStack() as ctx:
        consts = ctx.enter_context(tc.tile_pool(name="consts", bufs=1))

        def cload(name, shape, src, dt=f32r):
            t = consts.tile(shape, dt, tag=name, name=name)
            nc.sync.dma_start(t[:], src)
            return t

        # weights, split along the contraction (feature) axis where needed
        V_sb = [cload(f"V{i}", [128, KDIM], d["V"][128 * i : 128 * (i + 1), :]) for i in range(2)]
        Wf_sb = [cload(f"Wf{i}", [128, D1], d["Wf"][128 * i : 128 * (i + 1), :]) for i in range(2)]
        w1_sb = [cload(f"w1{i}", [128, D1], d["w1"][128 * i : 128 * (i + 1), :]) for i in range(2)]
        w2_sb = cload("w2", [D1, D2], d["w2"][:])
        w3_sb = cload("w3", [D2, D3], d["w3"][:])
        wout_sb = cload("wout", [D3, 1], d["wout"][:])
        wwide_sb = [cload(f"ww{i}", [128, 1], d["wwide"][128 * i : 128 * (i + 1), :]) for i in range(2)]
        b1_sb = cload("b1", [D1, 1], d["b1"][:], f32)
        b2_sb = cload("b2", [D2, 1], d["b2"][:], f32)
        b3_sb = cload("b3", [D3, 1], d["b3"][:], f32)
        bo_sb = cload("bo", [1, 1], d["bo"][:], f32)

        xp = ctx.enter_context(tc.tile_pool(name="xp", bufs=4))
        sqp = ctx.enter_context(tc.tile_pool(name="sqp", bufs=3))
        sp = ctx.enter_context(tc.tile_pool(name="sp", bufs=3))
        hp = ctx.enter_context(tc.tile_pool(name="hp", bufs=3))
        op = ctx.enter_context(tc.tile_pool(name="op", bufs=3))
        # PSUM: 8 banks total; budget = 3 + 2 + 1 + 1 + 1 = 8
        psB = ctx.enter_context(tc.tile_pool(name="psB", bufs=3, space="PSUM"))
        psH1 = ctx.enter_context(tc.tile_pool(name="psH1", bufs=2, space="PSUM"))
        psH2 = ctx.enter_context(tc.tile_pool(name="psH2", bufs=1, space="PSUM"))
        psH3 = ctx.enter_context(tc.tile_pool(name="psH3", bufs=1, space="PSUM"))
        psO = ctx.enter_context(tc.tile_pool(name="psO", bufs=1, space="PSUM"))

        mm = nc.tensor.matmul

        def relu_bias(out_ap, psum_ap, bias_t):
            # out = max(psum + bias, 0) — one DVE tensor_scalar op
            nc.vector.tensor_scalar(out_ap, psum_ap, bias_t[:], 0.0, Alu.add, Alu.max)

        for _rep in range(reps):
            for j in range(bl // bt):
                cols = bass.ts(j, bt)
                xt = [xp.tile([128, bt], f32r, tag=f"xt{i}", name=f"xt{i}") for i in range(2)]
                for i in range(2):
                    nc.sync.dma_start(xt[i][:], d["xT"][128 * i : 128 * (i + 1), cols])
                # x^2 on ACT
                sq = [sqp.tile([128, bt], f32r, tag=f"sq{i}", name=f"sq{i}") for i in range(2)]
                for i in range(2):
                    nc.scalar.square(sq[i][:], xt[i][:])

                s = []
                for ko in range(2):
                    kos = bass.ts(ko, 128)
                    pb = psB.tile([128, bt], f32, tag="pb", name="pb")  # xv chunk
                    mm(pb[:], V_sb[0][:, kos], xt[0][:], start=True, stop=False)
                    mm(pb[:], V_sb[1][:, kos], xt[1][:], start=False, stop=True)
                    sk = sp.tile([128, bt], f32r, tag=f"s{ko}", name=f"s{ko}")
                    nc.scalar.square(sk[:], pb[:])  # ACT: PSUM -> SBUF, xv^2
                    s.append(sk)

                # layer 1: ph1 = xv^2 @ w1' + x^2 @ W   (4 accumulating matmuls)
                ph1 = psH1.tile([D1, bt], f32, tag="ph1", name="ph1")
                mm(ph1[:], w1_sb[0][:], s[0][:], start=True, stop=False)
                mm(ph1[:], w1_sb[1][:], s[1][:], start=False, stop=False)
                mm(ph1[:], Wf_sb[0][:], sq[0][:], start=False, stop=False)
                mm(ph1[:], Wf_sb[1][:], sq[1][:], start=False, stop=True)
                h1 = hp.tile([D1, bt], f32r, tag="h1", name="h1")
                relu_bias(h1[:], ph1[:], b1_sb)

                ph2 = psH2.tile([D2, bt], f32, tag="ph2", name="ph2")
                mm(ph2[:], w2_sb[:], h1[:])
                h2 = hp.tile([D2, bt], f32r, tag="h2", name="h2")
                relu_bias(h2[:], ph2[:], b2_sb)

                ph3 = psH3.tile([D3, bt], f32, tag="ph3", name="ph3")
                mm(ph3[:], w3_sb[:], h2[:])
                h3 = hp.tile([D3, bt], f32r, tag="h3", name="h3")
                relu_bias(h3[:], ph3[:], b3_sb)

                # deep head + wide (linear) part accumulate into one [1, bt] psum
                po = psO.tile([1, bt], f32, tag="po", name="po")
                mm(po[:], wout_sb[:], h3[:], start=True, stop=False)
                mm(po[:], wwide_sb[0][:], xt[0][:], start=False, stop=False)
                mm(po[:], wwide_sb[1][:], xt[1][:], start=False, stop=True)
                ob = op.tile([1, bt], f32, tag="ob", name="ob")
                nc.scalar.activation(ob[:], po[:], AF.Sigmoid, bias=bo_sb[:])
                nc.sync.dma_start(outT[:, cols], ob[:])

    nc.compile()
    return nc


def make_in_maps(inputs, num_devices=NCORES, bl=BL):
    x = np.asarray(inputs["x"], np.float32)
    V = np.asarray(inputs["V"], np.float32)
    w1h = np.asarray(inputs["w1"], np.float32) * 0.5
    Wf = ((V.astype(np.float64) * V.astype(np.float64)) @ w1h.astype(np.float64)).astype(np.float32)
    common = {
        "V": np.ascontiguousarray(V),
        "Wf": np.ascontiguousarray(Wf),
        "w1": np.ascontiguousarray(w1h),
        "b1": np.asarray(inputs["b1"], np.float32).reshape(D1, 1),
        "w2": np.ascontiguousarray(np.asarray(inputs["w2"], np.float32)),
        "b2": np.asarray(inputs["b2"], np.float32).reshape(D2, 1),
        "w3": np.ascontiguousarray(np.asarray(inputs["w3"], np.float32)),
        "b3": np.asarray(inputs["b3"], np.float32).reshape(D3, 1),
        "wout": np.ascontiguousarray(np.asarray(inputs["w_out"], np.float32)),
        "wwide": np.ascontiguousarray(np.asarray(inputs["w_wide"], np.float32)),
        "bo": np.asarray(
            np.asarray(inputs["b_wide"], np.float32).reshape(()) +
            np.asarray(inputs["b_out"], np.float32).reshape(()), np.float32
        ).reshape(1, 1),
    }
    xT = np.ascontiguousarray(x.T)  # [F, BATCH]
    return [
        {**common, "xT": np.ascontiguousarray(xT[:, c * bl : (c + 1) * bl])}
        for c in range(num_devices)
    ]


_CACHED_NC = None


def _get_nc():
    global _CACHED_NC
    if _CACHED_NC is None:
        _CACHED_NC = build_program()
    return _CACHED_NC


def run(inputs, trace=False):
    nc = _get_nc()
    in_maps = make_in_maps(inputs)
    res = run_bass_kernel_spmd(nc, in_maps, core_ids=list(range(NCORES)), trace=trace)
    out = np.concatenate([r["outT"][0] for r in res.results])  # [BATCH]
    return out.reshape(BATCH, 1).astype(np.float32), res


def kernel(**inputs):
    out, _ = run(inputs, trace=bool(int(os.environ.get("NFM_TRACE", "0"))))
    return out
